# revision 6
# baseline (speedup 1.0000x reference)
"""DeepSeekMOE grouped-GEMM kernel for 8 Trainium2 NeuronCores.

Expert-parallel: core g handles expert group g.
Per core:  h = x @ w_up_gate ; act = silu(gate)*up ; out = act @ w_down
with x:[1536,2048], w_up_gate:[2048,2816], w_down:[1408,2048] (fp32).

Dataflow (transpose-free on device):
  - host supplies xT = x.T  ([2048,1536]) so both GEMM operands have the
    contraction dim on partitions.
  - GEMM1 computes hT tiles ([n_chunk 128, m 512]) = w1_colblock.T @ xT,
    so SwiGLU output actT lands directly in [E, M] layout — exactly the
    stationary-operand layout GEMM2 needs. out = actT.T @ w_down comes out
    in natural [M, H] orientation.
All matmuls run in float32r (TF32-class, 1 cycle/row on the PE array).

DMA queues: weights stream on the sync-engine HWDGE queue, xT and output
tiles on the scalar-engine HWDGE queue, so the first pair's weights are
not stuck behind the 12.6 MB xT load (queues execute in emission order).
"""

import sys
import numpy as np

if "/opt/trn_rl_repo" not in sys.path:
    sys.path.insert(0, "/opt/trn_rl_repo")

import concourse.bass as bass
import concourse.bacc as bacc
import concourse.mybir as mybir
import concourse.tile as tile
from concourse.bass_utils import run_bass_kernel_spmd

P = 128
M = 1536          # tokens per expert group
K = 2048          # hidden
N2 = 2816         # 2 * expert_dim (gate | up)
E = 1408          # expert_dim
H = 2048          # hidden (output)

KC = K // P       # 16 contraction chunks, GEMM1
EC = E // P       # 11 contraction chunks, GEMM2 / n-pairs
MT = 512          # m free-dim tile
NMT = M // MT     # 3 m-tiles
MC = M // P       # 12 output m-chunks, GEMM2
HT = 512          # h free-dim tile
NHT = H // HT     # 4 h-tiles

F32 = mybir.dt.float32
F32R = mybir.dt.float32r

_cache = {}


def _build_nc():
    nc = bacc.Bacc("TRN2", target_bir_lowering=False)

    xT = nc.declare_dram_parameter("xT", [K, M], F32R, isOutput=False)
    w1 = nc.declare_dram_parameter("w1", [K, N2], F32R, isOutput=False)
    w2 = nc.declare_dram_parameter("w2", [E, H], F32R, isOutput=False)
    out = nc.declare_dram_parameter("out", [M, H], F32, isOutput=True)

    def w1_colblock(col0):
        # [2048, 128] DRAM column block as a [128, 16, 128] partition-major AP
        return w1[:, col0:col0 + P].rearrange("(k p) c -> p k c", p=P)

    with tile.TileContext(nc) as tc:
        with tc.tile_pool(name="act", bufs=1) as act_pool, \
             tc.tile_pool(name="w2e0", bufs=1) as w2e0_pool, \
             tc.tile_pool(name="ps", bufs=8, space="PSUM") as ps_pool:
            # actT: [E, M] fp32r, resident through both phases (66 KB/part)
            act_t = [act_pool.tile([P, M], F32R, name=f"act{e}", tag=f"act{e}")
                     for e in range(EC)]
            # w2 e=0 chunk lives outside the xT-pool zone so its DMA can run
            # during phase 1 (no WAR dependency on the released xT space).
            w2t0 = w2e0_pool.tile([P, H], F32R, name="w2t0", tag="w2t0")

            # ---------------- Phase 1: GEMM1 + SwiGLU ----------------
            with tc.tile_pool(name="xt", bufs=1) as xt_pool:
                wgs, wus = {}, {}

                def load_pair_weights(i, pool):
                    wg = pool.tile([P, KC, P], F32R, name=f"wg{i}", tag="wg")
                    wu = pool.tile([P, KC, P], F32R, name=f"wu{i}", tag="wu")
                    nc.sync.dma_start(out=wg, in_=w1_colblock(i * P))
                    nc.sync.dma_start(out=wu, in_=w1_colblock(E + i * P))
                    wgs[i], wus[i] = wg, wu

                with tc.tile_pool(name="w1p", bufs=2) as w1_pool, \
                     tc.tile_pool(name="silu", bufs=2) as silu_pool:
                    # weights stream on the sync queue; pairs 0-2 first
                    load_pair_weights(0, w1_pool)
                    load_pair_weights(1, w1_pool)
                    load_pair_weights(2, w1_pool)

                    # xT streams on the scalar queue as [128, 512] m-slices
                    # in t-major order — exactly the order the startup MM
                    # block consumes them.
                    xts = [[None] * KC for _ in range(NMT)]
                    for t in range(NMT):
                        for k in range(KC):
                            xt = xt_pool.tile([P, MT], F32R,
                                              name=f"xt{t}_{k}",
                                              tag=f"xt{t}_{k}")
                            nc.scalar.dma_start(
                                out=xt,
                                in_=xT[k * P:(k + 1) * P,
                                       t * MT:(t + 1) * MT])
                            xts[t][k] = xt

                    def psum_pair(i, t):
                        g = ps_pool.tile([P, MT], F32, name=f"psg{i}_{t}",
                                         tag="ps")
                        u = ps_pool.tile([P, MT], F32, name=f"psu{i}_{t}",
                                         tag="ps")
                        return g, u

                    def swiglu(i, t, g, u):
                        tmp = silu_pool.tile([P, MT], F32,
                                             name=f"silu{i}_{t}", tag="silu")
                        nc.scalar.activation(
                            out=tmp, in_=g,
                            func=mybir.ActivationFunctionType.Silu)
                        nc.vector.tensor_mul(
                            out=act_t[i][:, t * MT:(t + 1) * MT],
                            in0=tmp, in1=u)

                    # startup block: pairs 0+1 interleaved, t-outer/k-inner,
                    # chasing the xT slice stream (4 PSUM banks per t-step)
                    for t in range(NMT):
                        pg0, pu0 = psum_pair(0, t)
                        pg1, pu1 = psum_pair(1, t)
                        for k in range(KC):
                            st, sp = (k == 0), (k == KC - 1)
                            xk = xts[t][k]
                            nc.tensor.matmul(pg0, wgs[0][:, k, :], xk,
                                             start=st, stop=sp)
                            nc.tensor.matmul(pu0, wus[0][:, k, :], xk,
                                             start=st, stop=sp)
                            nc.tensor.matmul(pg1, wgs[1][:, k, :], xk,
                                             start=st, stop=sp)
                            nc.tensor.matmul(pu1, wus[1][:, k, :], xk,
                                             start=st, stop=sp)
                        swiglu(0, t, pg0, pu0)
                        swiglu(1, t, pg1, pu1)
                    wgs.pop(0), wus.pop(0), wgs.pop(1), wus.pop(1)

                    # steady state: pairs 2..10
                    for i in range(2, EC):
                        if i + 1 < EC:
                            load_pair_weights(i + 1, w1_pool)
                        if i == 5:
                            # phase-2 head start: w2 e=0 (needed first)
                            nc.sync.dma_start(out=w2t0, in_=w2[0:P, :])
                        wg, wu = wgs.pop(i), wus.pop(i)
                        for t in range(NMT):
                            g, u = psum_pair(i, t)
                            for k in range(KC):
                                st, sp = (k == 0), (k == KC - 1)
                                xk = xts[t][k]
                                nc.tensor.matmul(g, wg[:, k, :], xk,
                                                 start=st, stop=sp)
                                nc.tensor.matmul(u, wu[:, k, :], xk,
                                                 start=st, stop=sp)
                            swiglu(i, t, g, u)

            # ---------------- Phase 2: GEMM2 ----------------
            with tc.tile_pool(name="w2p", bufs=1) as w2_pool, \
                 tc.tile_pool(name="ost", bufs=4) as out_pool:
                w2ts = [w2t0]
                for e in range(1, EC):
                    w2t = w2_pool.tile([P, H], F32R, name=f"w2t{e}",
                                       tag=f"w2t{e}")
                    # h-sliced loads: finer-grained deps so the first
                    # output chunks can start before the whole row lands
                    for h in range(NHT):
                        nc.sync.dma_start(
                            out=w2t[:, h * HT:(h + 1) * HT],
                            in_=w2[e * P:(e + 1) * P, h * HT:(h + 1) * HT])
                    w2ts.append(w2t)

                for mc in range(MC):
                    ps_o = [ps_pool.tile([P, HT], F32, name=f"pso{mc}_{h}",
                                         tag="ps") for h in range(NHT)]
                    for e in range(EC):
                        for h in range(NHT):
                            nc.tensor.matmul(
                                ps_o[h],
                                act_t[e][:, mc * P:(mc + 1) * P],
                                w2ts[e][:, h * HT:(h + 1) * HT],
                                start=(e == 0), stop=(e == EC - 1))
                    for h in range(NHT):
                        ot = out_pool.tile([P, HT], F32, name=f"ot{mc}_{h}",
                                           tag="ot")
                        nc.vector.tensor_copy(out=ot, in_=ps_o[h])
                        nc.scalar.dma_start(
                            out=out[mc * P:(mc + 1) * P, h * HT:(h + 1) * HT],
                            in_=ot)

    nc.compile()
    return nc


def kernel(x, w_up_gate, w_down):
    G = x.shape[0]
    if "nc" not in _cache:
        _cache["nc"] = _build_nc()
    nc = _cache["nc"]

    in_maps = []
    for g in range(G):
        in_maps.append({
            "xT": np.ascontiguousarray(x[g].T),
            "w1": np.ascontiguousarray(w_up_gate[g]),
            "w2": np.ascontiguousarray(w_down[g]),
        })
    res = run_bass_kernel_spmd(nc, in_maps, list(range(G)))
    return np.stack([res.results[g]["out"] for g in range(G)], axis=0)


# revision 8
# speedup vs baseline: 1.1180x; 1.1180x over previous
"""DeepSeekMOE grouped-GEMM kernel for 8 Trainium2 NeuronCores.

Expert-parallel: core g handles expert group g.
Per core:  h = x @ w_up_gate ; act = silu(gate)*up ; out = act @ w_down
with x:[1536,2048], w_up_gate:[2048,2816], w_down:[1408,2048] (fp32).

Dataflow (transpose-free on device):
  - host supplies xT = x.T  ([2048,1536]) so both GEMM operands have the
    contraction dim on partitions.
  - GEMM1 computes hT tiles ([n_chunk 128, m 512]) = w1_colblock.T @ xT,
    so SwiGLU output actT lands directly in [E, M] layout — exactly the
    stationary-operand layout GEMM2 needs. out = actT.T @ w_down comes out
    in natural [M, H] orientation.

Matmul inputs are bf16 (host-converted); accumulation stays fp32 in PSUM
and the output is fp32. Set IN_DT = F32R for a TF32-class variant (~2.6e-4
rel err instead of ~5e-3) at ~10% more runtime (doubled DMA traffic).

DMA queues: weights stream on the sync-engine HWDGE queue; xT, w_down and
output tiles on the scalar-engine HWDGE queue (queues execute in emission
order, so the first pair's weights are not stuck behind the xT load).
"""

import sys
import numpy as np

if "/opt/trn_rl_repo" not in sys.path:
    sys.path.insert(0, "/opt/trn_rl_repo")

import ml_dtypes
import concourse.bass as bass
import concourse.bacc as bacc
import concourse.mybir as mybir
import concourse.tile as tile
from concourse.bass_utils import run_bass_kernel_spmd

P = 128
M = 1536          # tokens per expert group
K = 2048          # hidden
N2 = 2816         # 2 * expert_dim (gate | up)
E = 1408          # expert_dim
H = 2048          # hidden (output)

KC = K // P       # 16 contraction chunks, GEMM1
EC = E // P       # 11 contraction chunks, GEMM2 / n-pairs
MT = 512          # m free-dim tile
NMT = M // MT     # 3 m-tiles
MC = M // P       # 12 output m-chunks, GEMM2
HT = 512          # h free-dim tile
NHT = H // HT     # 4 h-tiles

F32 = mybir.dt.float32
F32R = mybir.dt.float32r
BF16 = mybir.dt.bfloat16

IN_DT = BF16      # matmul input dtype (BF16 or F32R)

_cache = {}


def _np_in_dtype():
    return ml_dtypes.bfloat16 if IN_DT == BF16 else np.float32


def _build_nc():
    nc = bacc.Bacc("TRN2", target_bir_lowering=False)

    xT = nc.declare_dram_parameter("xT", [K, M], IN_DT, isOutput=False)
    w1 = nc.declare_dram_parameter("w1", [K, N2], IN_DT, isOutput=False)
    w2 = nc.declare_dram_parameter("w2", [E, H], IN_DT, isOutput=False)
    out = nc.declare_dram_parameter("out", [M, H], F32, isOutput=True)

    def w1_colblock(col0):
        # [2048, 128] DRAM column block as a [128, 16, 128] partition-major AP
        return w1[:, col0:col0 + P].rearrange("(k p) c -> p k c", p=P)

    with tile.TileContext(nc) as tc:
        with tc.tile_pool(name="act", bufs=1) as act_pool, \
             tc.tile_pool(name="w2p", bufs=1) as w2_pool, \
             tc.tile_pool(name="xt", bufs=1) as xt_pool, \
             tc.tile_pool(name="w1p", bufs=3) as w1_pool, \
             tc.tile_pool(name="silu", bufs=3) as silu_pool, \
             tc.tile_pool(name="ost", bufs=4) as out_pool, \
             tc.tile_pool(name="ps", bufs=8, space="PSUM") as ps_pool:

            # actT: [E, M], resident through both phases
            act_t = [act_pool.tile([P, M], IN_DT, name=f"act{e}", tag=f"act{e}")
                     for e in range(EC)]

            wgs, wus = {}, {}

            def load_pair_weights(i):
                wg = w1_pool.tile([P, KC, P], IN_DT, name=f"wg{i}", tag="wg")
                wu = w1_pool.tile([P, KC, P], IN_DT, name=f"wu{i}", tag="wu")
                nc.sync.dma_start(out=wg, in_=w1_colblock(i * P))
                nc.sync.dma_start(out=wu, in_=w1_colblock(E + i * P))
                wgs[i], wus[i] = wg, wu

            # ---------------- Phase 1: GEMM1 + SwiGLU ----------------
            # weights stream on the sync queue; pairs 0-2 first
            load_pair_weights(0)
            load_pair_weights(1)
            load_pair_weights(2)

            # xT streams on the scalar queue as [128, 512] m-slices in
            # t-major order — the order the startup MM block consumes them.
            xts = [[None] * KC for _ in range(NMT)]
            for t in range(NMT):
                for k in range(KC):
                    xt = xt_pool.tile([P, MT], IN_DT, name=f"xt{t}_{k}",
                                      tag=f"xt{t}_{k}")
                    nc.scalar.dma_start(
                        out=xt,
                        in_=xT[k * P:(k + 1) * P, t * MT:(t + 1) * MT])
                    xts[t][k] = xt

            # w_down preloads during phase 1 on the scalar queue (after xT)
            w2ts = []
            for e in range(EC):
                w2t = w2_pool.tile([P, H], IN_DT, name=f"w2t{e}", tag=f"w2t{e}")
                nc.scalar.dma_start(out=w2t, in_=w2[e * P:(e + 1) * P, :])
                w2ts.append(w2t)

            def psum_pair(i, t):
                g = ps_pool.tile([P, MT], F32, name=f"psg{i}_{t}", tag="ps")
                u = ps_pool.tile([P, MT], F32, name=f"psu{i}_{t}", tag="ps")
                return g, u

            def swiglu(i, t, g, u):
                tmp = silu_pool.tile([P, MT], F32, name=f"silu{i}_{t}",
                                     tag="silu")
                nc.scalar.activation(
                    out=tmp, in_=g, func=mybir.ActivationFunctionType.Silu)
                nc.vector.tensor_mul(
                    out=act_t[i][:, t * MT:(t + 1) * MT], in0=tmp, in1=u)

            # startup block: pairs 0+1 interleaved, t-outer/k-inner,
            # chasing the xT slice stream (4 PSUM banks per t-step)
            for t in range(NMT):
                pg0, pu0 = psum_pair(0, t)
                pg1, pu1 = psum_pair(1, t)
                for k in range(KC):
                    st, sp = (k == 0), (k == KC - 1)
                    xk = xts[t][k]
                    nc.tensor.matmul(pg0, wgs[0][:, k, :], xk, start=st, stop=sp)
                    nc.tensor.matmul(pu0, wus[0][:, k, :], xk, start=st, stop=sp)
                    nc.tensor.matmul(pg1, wgs[1][:, k, :], xk, start=st, stop=sp)
                    nc.tensor.matmul(pu1, wus[1][:, k, :], xk, start=st, stop=sp)
                swiglu(0, t, pg0, pu0)
                swiglu(1, t, pg1, pu1)
            wgs.pop(0), wus.pop(0), wgs.pop(1), wus.pop(1)

            # steady state: pairs 2..10
            for i in range(2, EC):
                if i + 1 < EC:
                    load_pair_weights(i + 1)
                wg, wu = wgs.pop(i), wus.pop(i)
                for t in range(NMT):
                    g, u = psum_pair(i, t)
                    for k in range(KC):
                        st, sp = (k == 0), (k == KC - 1)
                        xk = xts[t][k]
                        nc.tensor.matmul(g, wg[:, k, :], xk, start=st, stop=sp)
                        nc.tensor.matmul(u, wu[:, k, :], xk, start=st, stop=sp)
                    swiglu(i, t, g, u)

            # ---------------- Phase 2: GEMM2 ----------------
            for mc in range(MC):
                ps_o = [ps_pool.tile([P, HT], F32, name=f"pso{mc}_{h}",
                                     tag="ps") for h in range(NHT)]
                for e in range(EC):
                    for h in range(NHT):
                        nc.tensor.matmul(
                            ps_o[h],
                            act_t[e][:, mc * P:(mc + 1) * P],
                            w2ts[e][:, h * HT:(h + 1) * HT],
                            start=(e == 0), stop=(e == EC - 1))
                for h in range(NHT):
                    ot = out_pool.tile([P, HT], F32, name=f"ot{mc}_{h}",
                                       tag="ot")
                    nc.vector.tensor_copy(out=ot, in_=ps_o[h])
                    nc.scalar.dma_start(
                        out=out[mc * P:(mc + 1) * P, h * HT:(h + 1) * HT],
                        in_=ot)

    nc.compile()
    return nc


def kernel(x, w_up_gate, w_down):
    G = x.shape[0]
    if "nc" not in _cache:
        _cache["nc"] = _build_nc()
    nc = _cache["nc"]
    dt = _np_in_dtype()

    in_maps = []
    for g in range(G):
        in_maps.append({
            "xT": np.ascontiguousarray(x[g].T).astype(dt),
            "w1": np.ascontiguousarray(w_up_gate[g]).astype(dt),
            "w2": np.ascontiguousarray(w_down[g]).astype(dt),
        })
    res = run_bass_kernel_spmd(nc, in_maps, list(range(G)))
    return np.stack([res.results[g]["out"] for g in range(G)], axis=0)
